# revision 23
# baseline (speedup 1.0000x reference)
"""GPT-2 small forward pass on 8 Trainium2 NeuronCores (Bass/Tile).

ZERO-COLLECTIVE design: each 4-core group redundantly computes the full
12-layer trunk for its batch (cores 0-3 batch 0, cores 4-7 batch 1,
T=1024 tokens each), then each core computes the lm_head for its own
quarter vocab shard (4 x 12565 >= 50257). No cross-core communication at
all -- the 4x redundant trunk compute runs on cores that would otherwise
idle, and removes the per-layer K/V AllGathers (the dominant real-HW cost
of the previous sequence-parallel design).

Layout: activations feature-major xT [768, 1024] on-chip; scores computed
transposed (K stationary); PV uses token-major V with an appended ones
column so softmax denominators fall out of the same matmul. Causality on
the (otherwise idle) GpSimd engine AFTER exp: fully-invalid column spans
are memset to zero and the diagonal 128-wide span is multiplied by one
shared triangular 0/1 bf16 mask; fully-masked score blocks are skipped.
Attention runs per head-pair (q/k strips computed just-in-time);
fc->gelu->fcp interleave per hidden strip so the MLP hidden never
materializes fully.

Precision: bf16 matmul inputs, f32 PSUM/residual/LN math. Softmax without
max subtraction (scores bounded ~[-2.3, 2.7] for this model's init scale).
"""

import os
import sys

import numpy as np
import ml_dtypes

sys.path.insert(0, "/opt/trn_rl_repo")

import concourse.mybir as mybir  # noqa: E402
import concourse.tile as tile  # noqa: E402
from concourse import bacc  # noqa: E402
from concourse.bass_utils import run_bass_kernel_spmd  # noqa: E402

BF16 = mybir.dt.bfloat16
F32 = mybir.dt.float32
AF = mybir.ActivationFunctionType
ALU = mybir.AluOpType

P = 128
E = 768
EC = E // P  # 6
H = 12
HS = 64
B = 2
T = 1024
TB = T // P  # 8 token blocks
QH = 2  # 512-wide column halves
L_FULL = 12
V = 50257
NCORE = 8
VSH = 12565  # per-core vocab shard (4*12565 = 50260 >= V)
NVC = (VSH + 511) // 512  # 25 vocab chunks
EPS = 1e-5

_nbf = ml_dtypes.bfloat16


def _build(L, VS):
    from contextlib import ExitStack

    nc = bacc.Bacc("TRN2", target_bir_lowering=False, debug=False, num_devices=NCORE)

    # ---- DRAM I/O (per core; batch/shard selection is host-side) ----
    x0T_d = nc.dram_tensor("x0T", [E, T], F32, kind="ExternalInput").ap()
    lnf_d = nc.dram_tensor("lnfp", [2, P, EC], F32, kind="ExternalInput").ap()
    qkb_d = nc.dram_tensor("qkb", [L, P, 12], F32, kind="ExternalInput").ap()
    vbr_d = nc.dram_tensor("vbr", [L, E], BF16, kind="ExternalInput").ap()
    pbr_d = nc.dram_tensor("pbr", [L, E], BF16, kind="ExternalInput").ap()
    fcb_d = nc.dram_tensor("fcb", [L, P, 24], F32, kind="ExternalInput").ap()
    fpbr_d = nc.dram_tensor("fpbr", [L, E], BF16, kind="ExternalInput").ap()
    aw_d = nc.dram_tensor("aw", [L, E, 3 * E], BF16, kind="ExternalInput").ap()
    pw_d = nc.dram_tensor("pw", [L, E, E], BF16, kind="ExternalInput").ap()
    fw_d = nc.dram_tensor("fw", [L, E, 4 * E], BF16, kind="ExternalInput").ap()
    fpw_d = nc.dram_tensor("fpw", [L, 4 * E, E], BF16, kind="ExternalInput").ap()
    wteT_d = nc.dram_tensor("wteT", [E, VS], BF16, kind="ExternalInput").ap()
    msk_d = nc.dram_tensor("msk", [P, P], BF16, kind="ExternalInput").ap()
    out_d = nc.dram_tensor("out", [T, VS], F32, kind="ExternalOutput").ap()

    with tile.TileContext(nc) as tc:
        with ExitStack() as stack:
            pool_specs = [
                ("persist", 1, None),  # constants + masks
                ("resid", 1, None),    # xT f32
                ("lnp", 2, None),      # per-layer small params
                ("lnstage", 1, None),  # ln1/ln2/lnf [P,EC,T] bf16
                ("xbp", 3, None),      # xb/sq bf16 copies for LN stats
                ("smallp", 1, None),   # [1,T] stat rows
                ("awqk", 2, None),     # per-head-pair q/k weight strips
                ("awv", 7, None),      # v weight strips [P,768]
                ("pwp", 7, None),      # proj weight strips
                ("fwp", 8, None),      # fc weight strips
                ("fpwp", 4, None),     # fcp weight strips
                ("qkc", 3, None),      # per-head-pair qT/kT strips [P,T]
                ("vap", 1, None),      # va token-major tiles
                ("ep", 2, None),       # exp'd score tiles
                ("ytp", 1, None),      # yT
                ("htp", 4, None),      # transient hidden strips [P,512]
                ("tmpp", 2, None),     # LN tmp + zb
                ("wtp", 8, None),      # lm_head weight chunks
                ("lop", 3, None),      # lm_head output staging
                ("ps2", 3, "PSUM"),    # [P,1024] 2-bank psums
                ("ps1", 2, "PSUM"),    # [P,512] 1-bank psums
            ]
            pools = {}
            for pname, bufs, space in pool_specs:
                kw = {"name": pname, "bufs": bufs}
                if space:
                    kw["space"] = space
                pools[pname] = stack.enter_context(tc.tile_pool(**kw))
            (persist, resid, lnp, lnstage, xbp, smallp, awqk, awv, pwp, fwp,
             fpwp, qkc, vap, ep, ytp, htp, tmpp, wtp, lop, ps2, ps1) = (
                pools[n] for n, _, _ in pool_specs
            )

            # ---- persistent constants ----
            ones_col_f = persist.tile([P, 1], F32, name="ones_col_f")
            nc.vector.memset(ones_col_f, 1.0)
            ones_col = persist.tile([P, 1], BF16, name="ones_col")
            nc.vector.tensor_copy(ones_col, ones_col_f)
            ones_row_f = persist.tile([1, P], F32, name="ones_row_f")
            nc.vector.memset(ones_row_f, 1.0)
            ones_row = persist.tile([1, P], BF16, name="ones_row")
            nc.vector.tensor_copy(ones_row, ones_row_f)
            ones_512f = persist.tile([1, 512], F32, name="ones_512f")
            nc.vector.memset(ones_512f, 1.0)
            ones_512 = persist.tile([1, 512], BF16, name="ones_512")
            nc.vector.tensor_copy(ones_512, ones_512f)
            eps_sb = persist.tile([P, 1], F32, name="eps_sb")
            nc.vector.memset(eps_sb, EPS)
            mask_sb = persist.tile([P, P], BF16, name="mask_sb")
            nc.sync.dma_start(mask_sb, msk_d)
            lnfw = persist.tile([P, EC], F32, name="lnfw")
            lnfb = persist.tile([P, EC], F32, name="lnfb")
            nc.sync.dma_start(lnfw, lnf_d[0])
            nc.sync.dma_start(lnfb, lnf_d[1])

            # ---- residual (two half-tiles so qh-half deps stay independent) ----
            xTh = [resid.tile([P, EC, 512], F32, name=f"xT{qh}") for qh in range(QH)]
            for c in range(EC):
                for qh in range(QH):
                    nc.sync.dma_start(
                        xTh[qh][:, c, :],
                        x0T_d[c * P : (c + 1) * P, qh * 512 : (qh + 1) * 512],
                    )

            def layer_norm(w_sb, b_sb, tagp):
                """xT (f32, resid) -> new bf16 tile [P, EC, T].
                When w_sb is None the affine transform is folded into the
                consumer's weights/biases host-side and the normalized value
                is written directly by the second DVE op.

                Fully split per 512-column half so each half's chain only
                depends on that half of the residual (overlaps the previous
                phase's second half).
                """
                s1 = ps2.tile([1, T], F32, name=f"s1{tagp}", tag="ps2")
                s2 = ps2.tile([1, T], F32, name=f"s2{tagp}", tag="ps2")
                for c in range(EC):
                    for qh in range(QH):
                        sl = slice(qh * 512, (qh + 1) * 512)
                        xb = xbp.tile([P, 512], BF16, name=f"xb{tagp}", tag="xb")
                        sq = xbp.tile([P, 512], BF16, name=f"sq{tagp}", tag="sq")
                        nc.vector.tensor_copy(xb, xTh[qh][:, c, :])
                        nc.scalar.activation(sq, xTh[qh][:, c, :], AF.Square)
                        nc.tensor.matmul(
                            s1[:, sl], ones_col, xb,
                            start=(c == 0), stop=(c == EC - 1),
                        )
                        nc.tensor.matmul(
                            s2[:, sl], ones_col, sq,
                            start=(c == 0), stop=(c == EC - 1),
                        )
                rstd_b = smallp.tile([1, T], BF16, name=f"rstdb{tagp}", tag="smr")
                mr_b = smallp.tile([1, T], BF16, name=f"mrb{tagp}", tag="smb")
                a_sb = tmpp.tile([P, T], F32, name=f"asb{tagp}", tag="asb")
                c_sb = tmpp.tile([P, T], F32, name=f"csb{tagp}", tag="csb")
                for qh in range(QH):
                    sl = slice(qh * 512, (qh + 1) * 512)
                    var = smallp.tile([1, 512], F32, name=f"var{tagp}_{qh}",
                                      tag="smv")
                    mean = smallp.tile([1, 512], F32, name=f"mean{tagp}_{qh}",
                                       tag="smm")
                    nc.vector.tensor_scalar_mul(mean, s1[:, sl], 1.0 / E)
                    nc.vector.tensor_tensor(var, mean, mean, ALU.mult)
                    nc.vector.scalar_tensor_tensor(
                        var, s2[:, sl], 1.0 / E, var, ALU.mult, ALU.subtract,
                    )
                    nc.scalar.activation(var, var, AF.Sqrt, bias=eps_sb[0:1])
                    with nc.allow_low_precision(
                        reason="rstd feeds a bf16 broadcast matmul"
                    ):
                        nc.vector.reciprocal(rstd_b[:, sl], var)
                        # mr = (s1/E) * rstd
                        nc.vector.scalar_tensor_tensor(
                            mr_b[:, sl], s1[:, sl], 1.0 / E, rstd_b[:, sl],
                            ALU.mult, ALU.mult,
                        )
                    bc_ps = ps2.tile([P, T], F32, name=f"bc{tagp}_{qh}", tag="ps2")
                    nc.tensor.matmul(bc_ps[:, 0:512], ones_row, rstd_b[:, sl],
                                     start=True, stop=True)
                    nc.tensor.matmul(bc_ps[:, 512:1024], ones_row, mr_b[:, sl],
                                     start=True, stop=True)
                    nc.scalar.copy(a_sb[:, sl], bc_ps[:, 0:512])
                    nc.vector.tensor_copy(c_sb[:, sl], bc_ps[:, 512:1024])
                ln = lnstage.tile([P, EC, T], BF16, name=f"ln{tagp}", tag="ln")
                for c in range(EC):
                    for qh in range(QH):
                        sl = slice(qh * 512, (qh + 1) * 512)
                        eng = nc.vector if c % 2 == 0 else nc.gpsimd
                        tmp = tmpp.tile([P, 512], F32, name=f"lt{tagp}", tag="lntmp")
                        eng.tensor_tensor(tmp, xTh[qh][:, c, :], a_sb[:, sl],
                                          ALU.mult)
                        if w_sb is None:
                            with nc.allow_low_precision(
                                reason="normalized activation cast to bf16 for matmul"
                            ):
                                eng.tensor_tensor(ln[:, c, sl], tmp,
                                                  c_sb[:, sl], ALU.subtract)
                        else:
                            eng.tensor_tensor(tmp, tmp, c_sb[:, sl],
                                              ALU.subtract)
                            nc.scalar.activation(
                                ln[:, c, sl], tmp, AF.Identity,
                                bias=b_sb[:, c : c + 1], scale=w_sb[:, c : c + 1],
                            )
                return ln

            for l in range(L):
                # ---- layer params ----
                qkb = lnp.tile([P, 12], F32, name=f"qkb{l}", tag="qkb")
                nc.sync.dma_start(qkb, qkb_d[l])
                vbr = lnp.tile([1, E], BF16, name=f"vbr{l}", tag="vbr")
                nc.sync.dma_start(vbr, vbr_d[l : l + 1, :])

                ln1 = layer_norm(None, None, f"a{l}")

                # ---- v token-major va[tb]: [128 tok, H, HS+1] bf16 ----
                awv_sb = []
                for c in range(EC):
                    avc = awv.tile([P, E], BF16, name=f"awv{l}_{c}", tag="awv")
                    nc.sync.dma_start(avc, aw_d[l, c * P : (c + 1) * P, 2 * E :])
                    awv_sb.append(avc)
                va_t = [
                    vap.tile([P, H, HS + 1], BF16, name=f"va{l}_{tb}", tag=f"va{tb}")
                    for tb in range(TB)
                ]
                for tb in range(TB):
                    nc.gpsimd.memset(va_t[tb][:, :, HS : HS + 1], 1.0)
                    vps = ps2.tile([P, 1024], F32, name=f"v{l}_{tb}", tag="ps2")
                    tsl = slice(tb * P, (tb + 1) * P)
                    for c in range(EC):
                        nc.tensor.matmul(
                            vps[:, 0:512], ln1[:, c, tsl], awv_sb[c][:, 0:512],
                            start=(c == 0), stop=False,
                        )
                        nc.tensor.matmul(
                            vps[:, 512:768], ln1[:, c, tsl], awv_sb[c][:, 512:768],
                            start=(c == 0), stop=False,
                        )
                    nc.tensor.matmul(vps[:, 0:512], ones_row, vbr[:, 0:512],
                                     start=False, stop=True)
                    nc.tensor.matmul(vps[:, 512:768], ones_row, vbr[:, 512:768],
                                     start=False, stop=True)
                    nc.vector.tensor_copy(
                        va_t[tb][:, :, 0:HS],
                        vps[:, 0:768].rearrange("p (h d) -> p h d", d=HS),
                    )

                # ---- attention per head-pair ch: q/k strips -> per-head
                # scores -> exp -> mask -> PV -> /Z ----
                yT = ytp.tile([P, EC, T], BF16, name=f"yT{l}", tag="yT")
                blocks = {0: range(4), 1: range(8)}
                for ch in range(EC):
                    # q/k weight strips for this head pair: [P, EC, P] each
                    wq = awqk.tile([P, EC, P], BF16, name=f"wq{l}_{ch}", tag="wq")
                    wk = awqk.tile([P, EC, P], BF16, name=f"wk{l}_{ch}", tag="wk")
                    nc.sync.dma_start(
                        wq, aw_d[l][:, ch * P : (ch + 1) * P]
                        .rearrange("(c p) n -> p c n", p=P)
                    )
                    nc.sync.dma_start(
                        wk, aw_d[l][:, E + ch * P : E + (ch + 1) * P]
                        .rearrange("(c p) n -> p c n", p=P)
                    )
                    qTc = qkc.tile([P, T], BF16, name=f"qT{l}_{ch}", tag="qTc")
                    kTc = qkc.tile([P, T], BF16, name=f"kT{l}_{ch}", tag="kTc")
                    for wgt, dst, bofs in ((wq, qTc, 0), (wk, kTc, EC)):
                        pq = ps2.tile([P, T], F32, name=f"qk{l}_{ch}_{bofs}",
                                      tag="ps2")
                        for c in range(EC):
                            for qh in range(QH):
                                sl = slice(qh * 512, (qh + 1) * 512)
                                nc.tensor.matmul(
                                    pq[:, sl], wgt[:, c, :], ln1[:, c, sl],
                                    start=(c == 0), stop=(c == EC - 1),
                                )
                        for qh in range(QH):
                            sl = slice(qh * 512, (qh + 1) * 512)
                            with nc.allow_low_precision(
                                reason="q/k cast to bf16 for score matmuls"
                            ):
                                nc.vector.tensor_scalar_add(
                                    dst[:, sl], pq[:, sl],
                                    qkb[:, bofs + ch : bofs + ch + 1],
                                )
                    for hh in range(2):
                        h = 2 * ch + hh
                        po = hh * HS
                        Es = {}  # (qh, j) -> (tile, col_slice)
                        for qh in range(QH):
                            qsl = slice(qh * 512, (qh + 1) * 512)
                            js = list(blocks[qh])
                            for jp in range(0, len(js), 2):
                                scp = ps2.tile([P, 1024], F32,
                                               name=f"sc{l}_{h}_{qh}_{jp}",
                                               tag="ps2")
                                for k2, j in enumerate(js[jp : jp + 2]):
                                    nc.tensor.matmul(
                                        scp[:, k2 * 512 : (k2 + 1) * 512],
                                        kTc[po : po + HS, j * P : (j + 1) * P],
                                        qTc[po : po + HS, qsl],
                                        start=True, stop=True,
                                    )
                                et = ep.tile([P, 1024], BF16,
                                             name=f"E{l}_{h}_{qh}_{jp}",
                                             tag=f"E{jp}")
                                nc.scalar.activation(et, scp, AF.Exp)
                                for k2, j in enumerate(js[jp : jp + 2]):
                                    c0 = k2 * 512
                                    csl = slice(c0, c0 + 512)
                                    r = j * P - qh * 512
                                    if r >= 0:  # partially masked block
                                        if r > 0:
                                            nc.gpsimd.memset(
                                                et[:, c0 : c0 + r], 0.0)
                                        nc.gpsimd.tensor_tensor(
                                            et[:, c0 + r : c0 + r + P],
                                            et[:, c0 + r : c0 + r + P],
                                            mask_sb, ALU.mult,
                                        )
                                    Es[(qh, j)] = (et, csl)
                        zinv_b = smallp.tile([1, T], BF16, name=f"zib{l}_{h}",
                                             tag="zib")
                        for qh in range(QH):
                            js = list(blocks[qh])
                            yp = ps1.tile([P, 512], F32, name=f"y{l}_{h}_{qh}",
                                          tag="ps1")
                            for ji, j in enumerate(js):
                                et, csl = Es[(qh, j)]
                                nc.tensor.matmul(
                                    yp[0 : HS + 1, :], va_t[j][:, h, :],
                                    et[:, csl],
                                    start=(ji == 0), stop=(ji == len(js) - 1),
                                )
                            qsl = slice(qh * 512, (qh + 1) * 512)
                            with nc.allow_low_precision(
                                reason="1/Z in bf16 feeds a bf16 matmul anyway"
                            ):
                                nc.vector.reciprocal(zinv_b[:, qsl],
                                                     yp[HS : HS + 1, :])
                            zb_ps = ps1.tile([P, 512], F32, name=f"zp{l}_{h}_{qh}",
                                             tag="ps1")
                            nc.tensor.matmul(zb_ps[0:HS, :], ones_row[:, 0:HS],
                                             zinv_b[:, qsl], start=True,
                                             stop=True)
                            zb = tmpp.tile([HS, 512], F32, name=f"zb{l}_{h}",
                                           tag="zb")
                            nc.vector.tensor_copy(zb, zb_ps[0:HS, :])
                            nc.vector.tensor_tensor(
                                yT[po : po + HS, ch, qsl], yp[0:HS, :],
                                zb, ALU.mult,
                            )

                # ---- attn proj + residual ----
                pbr = lnp.tile([1, E], BF16, name=f"pbr{l}", tag="pbr")
                nc.sync.dma_start(pbr, pbr_d[l : l + 1, :])
                pw_sb = []
                for c in range(EC):
                    pwc = pwp.tile([P, E], BF16, name=f"pw{l}_{c}", tag="pw")
                    nc.sync.dma_start(pwc, pw_d[l, c * P : (c + 1) * P, :])
                    pw_sb.append(pwc)
                for co in range(EC):
                    ps = ps2.tile([P, T], F32, name=f"pj{l}_{co}", tag="ps2")
                    osl = slice(co * P, (co + 1) * P)
                    for c in range(EC):
                        for qh in range(QH):
                            sl = slice(qh * 512, (qh + 1) * 512)
                            nc.tensor.matmul(
                                ps[:, sl], pw_sb[c][:, osl], yT[:, c, sl],
                                start=(c == 0), stop=False,
                            )
                    for qh in range(QH):
                        sl = slice(qh * 512, (qh + 1) * 512)
                        nc.tensor.matmul(ps[:, sl], pbr[:, osl], ones_512,
                                         start=False, stop=True)
                        nc.vector.tensor_tensor(xTh[qh][:, co, :],
                                                xTh[qh][:, co, :], ps[:, sl],
                                                ALU.add)

                # ---- MLP: fc -> gelu -> fcp interleaved per hidden strip ----
                fcb = lnp.tile([P, 24], F32, name=f"fcb{l}", tag="fcb")
                nc.sync.dma_start(fcb, fcb_d[l])
                fpbr = lnp.tile([1, E], BF16, name=f"fpbr{l}", tag="fpbr")
                nc.sync.dma_start(fpbr, fpbr_d[l : l + 1, :])
                ln2 = layer_norm(None, None, f"b{l}")

                for qh in range(QH):
                    sl = slice(qh * 512, (qh + 1) * 512)
                    pss = [
                        ps2.tile([P, 1024], F32, name=f"fp{l}_{qh}_{cp}",
                                 tag="ps2")
                        for cp in range(3)
                    ]
                    for grp in range(4):
                        fw_sb = []
                        for c in range(EC):
                            fwc = fwp.tile([P, E], BF16,
                                           name=f"fw{l}_{qh}_{grp}_{c}", tag="fw")
                            nc.sync.dma_start(
                                fwc, fw_d[l, c * P : (c + 1) * P,
                                          grp * E : (grp + 1) * E]
                            )
                            fw_sb.append(fwc)
                        for t in range(EC):
                            hc = grp * EC + t
                            hps = ps1.tile([P, 512], F32, name=f"fc{l}_{qh}_{hc}",
                                           tag="ps1")
                            for c in range(EC):
                                nc.tensor.matmul(
                                    hps, fw_sb[c][:, t * P : (t + 1) * P],
                                    ln2[:, c, sl],
                                    start=(c == 0), stop=(c == EC - 1),
                                )
                            ht = htp.tile([P, 512], BF16, name=f"hT{l}_{qh}_{hc}",
                                          tag="hT")
                            nc.scalar.activation(ht, hps, AF.Gelu,
                                                 bias=fcb[:, hc : hc + 1])
                            fpc = fpwp.tile([P, E], BF16,
                                            name=f"fpw{l}_{qh}_{hc}", tag="fpw")
                            nc.sync.dma_start(fpc,
                                              fpw_d[l, hc * P : (hc + 1) * P, :])
                            for co in range(EC):
                                nc.tensor.matmul(
                                    pss[co // 2][:, (co % 2) * 512
                                                 : (co % 2 + 1) * 512],
                                    fpc[:, co * P : (co + 1) * P], ht,
                                    start=(hc == 0), stop=False,
                                )
                    for co in range(EC):
                        psl = slice((co % 2) * 512, (co % 2 + 1) * 512)
                        nc.tensor.matmul(
                            pss[co // 2][:, psl],
                            fpbr[:, co * P : (co + 1) * P],
                            ones_512, start=False, stop=True,
                        )
                        nc.vector.tensor_tensor(
                            xTh[qh][:, co, :], xTh[qh][:, co, :],
                            pss[co // 2][:, psl],
                            ALU.add,
                        )

            # ---- final LN + lm_head (own vocab shard, all T tokens) ----
            lnf = layer_norm(lnfw, lnfb, "f")
            for vc in range(NVC):
                w = min(512, VS - vc * 512)
                wts = []
                for c in range(EC):
                    wtc = wtp.tile([P, 512], BF16, name=f"wt{vc}_{c}", tag="wt")
                    nc.sync.dma_start(
                        wtc[:, :w],
                        wteT_d[c * P : (c + 1) * P, vc * 512 : vc * 512 + w],
                    )
                    wts.append(wtc)
                for tb in range(TB):
                    if tb % 2 == 0:
                        lps = ps2.tile([P, 1024], F32, name=f"lm{vc}_{tb}",
                                       tag="ps2")
                    psl = slice((tb % 2) * 512, (tb % 2) * 512 + w)
                    for c in range(EC):
                        nc.tensor.matmul(
                            lps[:, psl],
                            lnf[:, c, tb * P : (tb + 1) * P],
                            wts[c][:, :w],
                            start=(c == 0), stop=(c == EC - 1),
                        )
                    o = lop.tile([P, 512], F32, name=f"lo{vc}_{tb}", tag="lo")
                    if tb % 2 == 0:
                        nc.scalar.copy(o[:, :w], lps[:, psl])
                    else:
                        nc.vector.tensor_copy(o[:, :w], lps[:, psl])
                    nc.sync.dma_start(
                        out_d[tb * P : (tb + 1) * P, vc * 512 : vc * 512 + w],
                        o[:, :w],
                    )

    nc.compile()
    return nc


_CACHE = {}


def _get_nc(L, VS):
    key = (L, VS)
    if key not in _CACHE:
        _CACHE[key] = _build(L, VS)
    return _CACHE[key]


def _bf(a):
    return np.ascontiguousarray(a.astype(_nbf))


def _pp(a, cols):
    """[L?, n*128] feature vector -> per-partition layout [..., 128, n]."""
    a = np.asarray(a, np.float32)
    shp = a.shape[:-1]
    n = a.shape[-1] // P
    return np.ascontiguousarray(a.reshape(*shp, n, P).swapaxes(-1, -2))


def _prepare(inputs, L, VS):
    """Host prep: embedding, weight cast/fold/transpose, per-core in_maps."""
    idx = np.asarray(inputs["idx"])
    wte = np.asarray(inputs["wte"], np.float32)
    wpe = np.asarray(inputs["wpe"], np.float32)

    x0 = wte[idx] + wpe[None, :T]  # [B, T, E] f32

    ln1_w = np.asarray(inputs["ln1_w"], np.float32)[:L]
    ln1_b = np.asarray(inputs["ln1_b"], np.float32)[:L]
    ln2_w = np.asarray(inputs["ln2_w"], np.float32)[:L]
    ln2_b = np.asarray(inputs["ln2_b"], np.float32)[:L]

    attn_w = np.asarray(inputs["attn_w"], np.float32)[:L].copy()
    attn_b = np.asarray(inputs["attn_b"], np.float32)[:L].copy()
    scale = 1.0 / np.sqrt(HS)
    attn_w[:, :, :E] *= scale
    attn_b[:, :E] *= scale
    # fold ln1 affine: ln(x)@W + b == n(x)@(w*W) + (b + ln_b@W)
    attn_b = attn_b + np.einsum("le,leo->lo", ln1_b, attn_w)
    attn_w = attn_w * ln1_w[:, :, None]

    fc_w = np.asarray(inputs["fc_w"], np.float32)[:L].copy()
    fc_b = np.asarray(inputs["fc_b"], np.float32)[:L].copy()
    fc_b = fc_b + np.einsum("le,leo->lo", ln2_b, fc_w)
    fc_w = fc_w * ln2_w[:, :, None]

    aw = _bf(attn_w)
    pw = _bf(np.asarray(inputs["proj_w"], np.float32)[:L])
    fw = _bf(fc_w)
    fpw = _bf(np.asarray(inputs["fcp_w"], np.float32)[:L])

    lnfp = np.stack(
        [_pp(np.asarray(inputs["lnf_w"], np.float32), EC),
         _pp(np.asarray(inputs["lnf_b"], np.float32), EC)], axis=0
    )
    qkb = _pp(attn_b[:, : 2 * E], 12)
    vbr = _bf(attn_b[:, 2 * E :])
    pbr = _bf(np.asarray(inputs["proj_b"], np.float32)[:L])
    fcb = _pp(fc_b, 24)
    fpbr = _bf(np.asarray(inputs["fcp_b"], np.float32)[:L])

    # wteT padded + per-core vocab shards (4 shards across each batch group)
    wteT = np.zeros((E, 4 * VS), _nbf)
    nv = min(V, 4 * VS)
    wteT[:, :nv] = _bf(wte.T[:, :nv])

    # multiplicative causal mask for the diagonal 128x128 score block
    kpos = np.arange(P)
    msk = (kpos[:, None] <= kpos[None, :]).astype(_nbf)

    in_maps = []
    for c in range(NCORE):
        b = c // 4
        sh = c % 4
        x0T = np.ascontiguousarray(x0[b].T)  # [768, 1024]
        in_maps.append(
            {
                "x0T": x0T,
                "lnfp": lnfp,
                "qkb": qkb, "vbr": vbr, "pbr": pbr, "fcb": fcb, "fpbr": fpbr,
                "aw": aw, "pw": pw, "fw": fw, "fpw": fpw,
                "wteT": np.ascontiguousarray(wteT[:, sh * VS : (sh + 1) * VS]),
                "msk": msk,
            }
        )
    return in_maps


def _run(inputs, L, VS, trace=False):
    nc = _get_nc(L, VS)
    in_maps = _prepare(inputs, L, VS)
    res = run_bass_kernel_spmd(
        nc, in_maps, core_ids=list(range(NCORE)), trace=trace
    )
    # core c holds batch c//4, vocab shard c%4: concat shards per batch
    outs = [res.results[c]["out"] for c in range(NCORE)]
    logits = np.stack(
        [np.concatenate(outs[4 * b : 4 * b + 4], axis=1)[:, :V] for b in range(B)]
    )
    return np.ascontiguousarray(logits), res


def kernel(**inputs) -> np.ndarray:
    trace = bool(os.environ.get("_KERNEL_TRACE"))
    logits, _ = _run(inputs, L_FULL, VSH, trace=trace)
    return logits


if __name__ == "__main__":
    pass


# revision 24
# speedup vs baseline: 1.2578x; 1.2578x over previous
"""GPT-2 small forward pass on 8 Trainium2 NeuronCores (Bass/Tile).

ZERO-COLLECTIVE design: each 4-core group redundantly computes the full
12-layer trunk for its batch (cores 0-3 batch 0, cores 4-7 batch 1,
T=1024 tokens each), then each core computes the lm_head for its own
quarter vocab shard (4 x 12565 >= 50257). No cross-core communication at
all -- the 4x redundant trunk compute runs on cores that would otherwise
idle, and removes the per-layer K/V AllGathers (the dominant real-HW cost
of the previous sequence-parallel design).

Layout: activations feature-major xT [768, 1024] on-chip; scores computed
transposed (K stationary); PV uses token-major V with an appended ones
column so softmax denominators fall out of the same matmul. Causality on
the (otherwise idle) GpSimd engine AFTER exp: fully-invalid column spans
are memset to zero and the diagonal 128-wide span is multiplied by one
shared triangular 0/1 bf16 mask; fully-masked score blocks are skipped.
Attention runs per head-pair (q/k strips computed just-in-time);
fc->gelu->fcp interleave per hidden strip so the MLP hidden never
materializes fully.

Precision: bf16 matmul inputs, f32 PSUM/residual/LN math. Softmax without
max subtraction (scores bounded ~[-2.3, 2.7] for this model's init scale).
"""

import os
import sys

import numpy as np
import ml_dtypes

sys.path.insert(0, "/opt/trn_rl_repo")

import concourse.mybir as mybir  # noqa: E402
import concourse.tile as tile  # noqa: E402
from concourse import bacc  # noqa: E402
from concourse.bass_utils import run_bass_kernel_spmd  # noqa: E402

BF16 = mybir.dt.bfloat16
F32 = mybir.dt.float32
AF = mybir.ActivationFunctionType
ALU = mybir.AluOpType

P = 128
E = 768
EC = E // P  # 6
H = 12
HS = 64
B = 2
T = 1024
TB = T // P  # 8 token blocks
QH = 2  # 512-wide column halves
L_FULL = 12
V = 50257
NCORE = 8
VSH = 12565  # per-core vocab shard (4*12565 = 50260 >= V)
NVC = (VSH + 511) // 512  # 25 vocab chunks
EPS = 1e-5

_nbf = ml_dtypes.bfloat16


def _build(L, VS, no_bias=False):
    from contextlib import ExitStack

    nc = bacc.Bacc("TRN2", target_bir_lowering=False, debug=False, num_devices=NCORE)

    # ---- DRAM I/O (per core; batch/shard selection is host-side) ----
    x0T_d = nc.dram_tensor("x0T", [E, T], F32, kind="ExternalInput").ap()
    lnf_d = nc.dram_tensor("lnfp", [2, P, EC], F32, kind="ExternalInput").ap()
    qkb_d = nc.dram_tensor("qkb", [L, P, 12], F32, kind="ExternalInput").ap()
    vbr_d = nc.dram_tensor("vbr", [L, E], BF16, kind="ExternalInput").ap()
    pbr_d = nc.dram_tensor("pbr", [L, E], BF16, kind="ExternalInput").ap()
    fcb_d = nc.dram_tensor("fcb", [L, P, 24], F32, kind="ExternalInput").ap()
    fpbr_d = nc.dram_tensor("fpbr", [L, E], BF16, kind="ExternalInput").ap()
    aw_d = nc.dram_tensor("aw", [L, E, 3 * E], BF16, kind="ExternalInput").ap()
    pw_d = nc.dram_tensor("pw", [L, E, E], BF16, kind="ExternalInput").ap()
    fw_d = nc.dram_tensor("fw", [L, E, 4 * E], BF16, kind="ExternalInput").ap()
    fpw_d = nc.dram_tensor("fpw", [L, 4 * E, E], BF16, kind="ExternalInput").ap()
    wteT_d = nc.dram_tensor("wteT", [E, VS], BF16, kind="ExternalInput").ap()
    msk_d = nc.dram_tensor("msk", [P, P], BF16, kind="ExternalInput").ap()
    out_d = nc.dram_tensor("out", [T, VS], F32, kind="ExternalOutput").ap()

    with tile.TileContext(nc) as tc:
        with ExitStack() as stack:
            pool_specs = [
                ("persist", 1, None),  # constants + masks
                ("resid", 1, None),    # xT f32
                ("lnp", 2, None),      # per-layer small params
                ("lnstage", 1, None),  # ln1/ln2/lnf [P,EC,T] bf16
                ("xbp", 3, None),      # xb/sq bf16 copies for LN stats
                ("smallp", 1, None),   # [1,T] stat rows
                ("awqk", 2, None),     # per-head-pair q/k weight strips
                ("awv", 7, None),      # v weight strips [P,768]
                ("pwp", 7, None),      # proj weight strips
                ("fwp", 8, None),      # fc weight strips
                ("fpwp", 4, None),     # fcp weight strips
                ("qkc", 3, None),      # per-head-pair qT/kT strips [P,T]
                ("vap", 1, None),      # va token-major tiles
                ("ep", 2, None),       # exp'd score tiles
                ("ytp", 1, None),      # yT
                ("htp", 4, None),      # transient hidden strips [P,512]
                ("tmpp", 2, None),     # LN tmp + zb
                ("wtp", 8, None),      # lm_head weight chunks
                ("lop", 3, None),      # lm_head output staging
                ("ps2", 3, "PSUM"),    # [P,1024] 2-bank psums
                ("ps1", 2, "PSUM"),    # [P,512] 1-bank psums
            ]
            pools = {}
            for pname, bufs, space in pool_specs:
                kw = {"name": pname, "bufs": bufs}
                if space:
                    kw["space"] = space
                pools[pname] = stack.enter_context(tc.tile_pool(**kw))
            (persist, resid, lnp, lnstage, xbp, smallp, awqk, awv, pwp, fwp,
             fpwp, qkc, vap, ep, ytp, htp, tmpp, wtp, lop, ps2, ps1) = (
                pools[n] for n, _, _ in pool_specs
            )

            # ---- persistent constants ----
            ones_col_f = persist.tile([P, 1], F32, name="ones_col_f")
            nc.vector.memset(ones_col_f, 1.0)
            ones_col = persist.tile([P, 1], BF16, name="ones_col")
            nc.vector.tensor_copy(ones_col, ones_col_f)
            ones_row_f = persist.tile([1, P], F32, name="ones_row_f")
            nc.vector.memset(ones_row_f, 1.0)
            ones_row = persist.tile([1, P], BF16, name="ones_row")
            nc.vector.tensor_copy(ones_row, ones_row_f)
            ones_512f = persist.tile([1, 512], F32, name="ones_512f")
            nc.vector.memset(ones_512f, 1.0)
            ones_512 = persist.tile([1, 512], BF16, name="ones_512")
            nc.vector.tensor_copy(ones_512, ones_512f)
            eps_sb = persist.tile([P, 1], F32, name="eps_sb")
            nc.vector.memset(eps_sb, EPS)
            mask_sb = persist.tile([P, P], BF16, name="mask_sb")
            nc.sync.dma_start(mask_sb, msk_d)
            lnfw = persist.tile([P, EC], F32, name="lnfw")
            lnfb = persist.tile([P, EC], F32, name="lnfb")
            nc.sync.dma_start(lnfw, lnf_d[0])
            nc.sync.dma_start(lnfb, lnf_d[1])

            # ---- residual (two half-tiles so qh-half deps stay independent) ----
            xTh = [resid.tile([P, EC, 512], F32, name=f"xT{qh}") for qh in range(QH)]
            for c in range(EC):
                for qh in range(QH):
                    nc.sync.dma_start(
                        xTh[qh][:, c, :],
                        x0T_d[c * P : (c + 1) * P, qh * 512 : (qh + 1) * 512],
                    )

            def layer_norm(w_sb, b_sb, tagp):
                """xT (f32, resid) -> new bf16 tile [P, EC, T].
                When w_sb is None the affine transform is folded into the
                consumer's weights/biases host-side and the normalized value
                is written directly by the second DVE op.

                Fully split per 512-column half so each half's chain only
                depends on that half of the residual (overlaps the previous
                phase's second half).
                """
                s1 = ps2.tile([1, T], F32, name=f"s1{tagp}", tag="ps2")
                s2 = ps2.tile([1, T], F32, name=f"s2{tagp}", tag="ps2")
                for c in range(EC):
                    for qh in range(QH):
                        sl = slice(qh * 512, (qh + 1) * 512)
                        xb = xbp.tile([P, 512], BF16, name=f"xb{tagp}", tag="xb")
                        sq = xbp.tile([P, 512], BF16, name=f"sq{tagp}", tag="sq")
                        nc.vector.tensor_copy(xb, xTh[qh][:, c, :])
                        nc.scalar.activation(sq, xTh[qh][:, c, :], AF.Square)
                        nc.tensor.matmul(
                            s1[:, sl], ones_col, xb,
                            start=(c == 0), stop=(c == EC - 1),
                        )
                        nc.tensor.matmul(
                            s2[:, sl], ones_col, sq,
                            start=(c == 0), stop=(c == EC - 1),
                        )
                rstd_b = smallp.tile([1, T], BF16, name=f"rstdb{tagp}", tag="smr")
                mr_b = smallp.tile([1, T], BF16, name=f"mrb{tagp}", tag="smb")
                a_sb = tmpp.tile([P, T], F32, name=f"asb{tagp}", tag="asb")
                c_sb = tmpp.tile([P, T], F32, name=f"csb{tagp}", tag="csb")
                for qh in range(QH):
                    sl = slice(qh * 512, (qh + 1) * 512)
                    var = smallp.tile([1, 512], F32, name=f"var{tagp}_{qh}",
                                      tag="smv")
                    mean = smallp.tile([1, 512], F32, name=f"mean{tagp}_{qh}",
                                       tag="smm")
                    nc.vector.tensor_scalar_mul(mean, s1[:, sl], 1.0 / E)
                    nc.vector.tensor_tensor(var, mean, mean, ALU.mult)
                    nc.vector.scalar_tensor_tensor(
                        var, s2[:, sl], 1.0 / E, var, ALU.mult, ALU.subtract,
                    )
                    nc.scalar.activation(var, var, AF.Sqrt, bias=eps_sb[0:1])
                    with nc.allow_low_precision(
                        reason="rstd feeds a bf16 broadcast matmul"
                    ):
                        nc.vector.reciprocal(rstd_b[:, sl], var)
                        # mr = (s1/E) * rstd
                        nc.vector.scalar_tensor_tensor(
                            mr_b[:, sl], s1[:, sl], 1.0 / E, rstd_b[:, sl],
                            ALU.mult, ALU.mult,
                        )
                    bc_ps = ps2.tile([P, T], F32, name=f"bc{tagp}_{qh}", tag="ps2")
                    nc.tensor.matmul(bc_ps[:, 0:512], ones_row, rstd_b[:, sl],
                                     start=True, stop=True)
                    nc.tensor.matmul(bc_ps[:, 512:1024], ones_row, mr_b[:, sl],
                                     start=True, stop=True)
                    nc.scalar.copy(a_sb[:, sl], bc_ps[:, 0:512])
                    nc.vector.tensor_copy(c_sb[:, sl], bc_ps[:, 512:1024])
                ln = lnstage.tile([P, EC, T], BF16, name=f"ln{tagp}", tag="ln")
                for c in range(EC):
                    for qh in range(QH):
                        sl = slice(qh * 512, (qh + 1) * 512)
                        eng = nc.vector if c % 2 == 0 else nc.gpsimd
                        tmp = tmpp.tile([P, 512], F32, name=f"lt{tagp}", tag="lntmp")
                        eng.tensor_tensor(tmp, xTh[qh][:, c, :], a_sb[:, sl],
                                          ALU.mult)
                        if w_sb is None:
                            with nc.allow_low_precision(
                                reason="normalized activation cast to bf16 for matmul"
                            ):
                                eng.tensor_tensor(ln[:, c, sl], tmp,
                                                  c_sb[:, sl], ALU.subtract)
                        else:
                            eng.tensor_tensor(tmp, tmp, c_sb[:, sl],
                                              ALU.subtract)
                            nc.scalar.activation(
                                ln[:, c, sl], tmp, AF.Identity,
                                bias=b_sb[:, c : c + 1], scale=w_sb[:, c : c + 1],
                            )
                return ln

            for l in range(L):
                # ---- layer params ----
                qkb = lnp.tile([P, 12], F32, name=f"qkb{l}", tag="qkb")
                nc.sync.dma_start(qkb, qkb_d[l])
                vbr = lnp.tile([1, E], BF16, name=f"vbr{l}", tag="vbr")
                nc.sync.dma_start(vbr, vbr_d[l : l + 1, :])

                ln1 = layer_norm(None, None, f"a{l}")

                # ---- v token-major va[tb]: [128 tok, H, HS+1] bf16 ----
                awv_sb = []
                for c in range(EC):
                    avc = awv.tile([P, E], BF16, name=f"awv{l}_{c}", tag="awv")
                    nc.sync.dma_start(avc, aw_d[l, c * P : (c + 1) * P, 2 * E :])
                    awv_sb.append(avc)
                va_t = [
                    vap.tile([P, H, HS + 1], BF16, name=f"va{l}_{tb}", tag=f"va{tb}")
                    for tb in range(TB)
                ]
                for tb in range(TB):
                    nc.gpsimd.memset(va_t[tb][:, :, HS : HS + 1], 1.0)
                    vps = ps2.tile([P, 1024], F32, name=f"v{l}_{tb}", tag="ps2")
                    tsl = slice(tb * P, (tb + 1) * P)
                    for c in range(EC):
                        last = no_bias and c == EC - 1
                        nc.tensor.matmul(
                            vps[:, 0:512], ln1[:, c, tsl], awv_sb[c][:, 0:512],
                            start=(c == 0), stop=last,
                        )
                        nc.tensor.matmul(
                            vps[:, 512:768], ln1[:, c, tsl], awv_sb[c][:, 512:768],
                            start=(c == 0), stop=last,
                        )
                    if not no_bias:
                        nc.tensor.matmul(vps[:, 0:512], ones_row, vbr[:, 0:512],
                                         start=False, stop=True)
                        nc.tensor.matmul(vps[:, 512:768], ones_row,
                                         vbr[:, 512:768], start=False, stop=True)
                    nc.vector.tensor_copy(
                        va_t[tb][:, :, 0:HS],
                        vps[:, 0:768].rearrange("p (h d) -> p h d", d=HS),
                    )

                # ---- attention per head-pair ch: q/k strips -> per-head
                # scores -> exp -> mask -> PV -> /Z ----
                yT = ytp.tile([P, EC, T], BF16, name=f"yT{l}", tag="yT")
                blocks = {0: range(4), 1: range(8)}
                for ch in range(EC):
                    # q/k weight strips for this head pair: [P, EC, P] each
                    wq = awqk.tile([P, EC, P], BF16, name=f"wq{l}_{ch}", tag="wq")
                    wk = awqk.tile([P, EC, P], BF16, name=f"wk{l}_{ch}", tag="wk")
                    nc.sync.dma_start(
                        wq, aw_d[l][:, ch * P : (ch + 1) * P]
                        .rearrange("(c p) n -> p c n", p=P)
                    )
                    nc.sync.dma_start(
                        wk, aw_d[l][:, E + ch * P : E + (ch + 1) * P]
                        .rearrange("(c p) n -> p c n", p=P)
                    )
                    qTc = qkc.tile([P, T], BF16, name=f"qT{l}_{ch}", tag="qTc")
                    kTc = qkc.tile([P, T], BF16, name=f"kT{l}_{ch}", tag="kTc")
                    for wgt, dst, bofs in ((wq, qTc, 0), (wk, kTc, EC)):
                        pq = ps2.tile([P, T], F32, name=f"qk{l}_{ch}_{bofs}",
                                      tag="ps2")
                        for c in range(EC):
                            for qh in range(QH):
                                sl = slice(qh * 512, (qh + 1) * 512)
                                nc.tensor.matmul(
                                    pq[:, sl], wgt[:, c, :], ln1[:, c, sl],
                                    start=(c == 0), stop=(c == EC - 1),
                                )
                        for qh in range(QH):
                            sl = slice(qh * 512, (qh + 1) * 512)
                            with nc.allow_low_precision(
                                reason="q/k cast to bf16 for score matmuls"
                            ):
                                nc.vector.tensor_scalar_add(
                                    dst[:, sl], pq[:, sl],
                                    qkb[:, bofs + ch : bofs + ch + 1],
                                )
                    for hh in range(2):
                        h = 2 * ch + hh
                        po = hh * HS
                        Es = {}  # (qh, j) -> (tile, col_slice)
                        for qh in range(QH):
                            qsl = slice(qh * 512, (qh + 1) * 512)
                            js = list(blocks[qh])
                            for jp in range(0, len(js), 2):
                                scp = ps2.tile([P, 1024], F32,
                                               name=f"sc{l}_{h}_{qh}_{jp}",
                                               tag="ps2")
                                for k2, j in enumerate(js[jp : jp + 2]):
                                    nc.tensor.matmul(
                                        scp[:, k2 * 512 : (k2 + 1) * 512],
                                        kTc[po : po + HS, j * P : (j + 1) * P],
                                        qTc[po : po + HS, qsl],
                                        start=True, stop=True,
                                    )
                                et = ep.tile([P, 1024], BF16,
                                             name=f"E{l}_{h}_{qh}_{jp}",
                                             tag=f"E{jp}")
                                nc.scalar.activation(et, scp, AF.Exp)
                                for k2, j in enumerate(js[jp : jp + 2]):
                                    c0 = k2 * 512
                                    csl = slice(c0, c0 + 512)
                                    r = j * P - qh * 512
                                    if r >= 0:  # partially masked block
                                        if r > 0:
                                            nc.gpsimd.memset(
                                                et[:, c0 : c0 + r], 0.0)
                                        nc.gpsimd.tensor_tensor(
                                            et[:, c0 + r : c0 + r + P],
                                            et[:, c0 + r : c0 + r + P],
                                            mask_sb, ALU.mult,
                                        )
                                    Es[(qh, j)] = (et, csl)
                        zinv_b = smallp.tile([1, T], BF16, name=f"zib{l}_{h}",
                                             tag="zib")
                        for qh in range(QH):
                            js = list(blocks[qh])
                            yp = ps1.tile([P, 512], F32, name=f"y{l}_{h}_{qh}",
                                          tag="ps1")
                            for ji, j in enumerate(js):
                                et, csl = Es[(qh, j)]
                                nc.tensor.matmul(
                                    yp[0 : HS + 1, :], va_t[j][:, h, :],
                                    et[:, csl],
                                    start=(ji == 0), stop=(ji == len(js) - 1),
                                )
                            qsl = slice(qh * 512, (qh + 1) * 512)
                            with nc.allow_low_precision(
                                reason="1/Z in bf16 feeds a bf16 matmul anyway"
                            ):
                                nc.vector.reciprocal(zinv_b[:, qsl],
                                                     yp[HS : HS + 1, :])
                            zb_ps = ps1.tile([P, 512], F32, name=f"zp{l}_{h}_{qh}",
                                             tag="ps1")
                            nc.tensor.matmul(zb_ps[0:HS, :], ones_row[:, 0:HS],
                                             zinv_b[:, qsl], start=True,
                                             stop=True)
                            zb = tmpp.tile([HS, 512], F32, name=f"zb{l}_{h}",
                                           tag="zb")
                            nc.vector.tensor_copy(zb, zb_ps[0:HS, :])
                            nc.vector.tensor_tensor(
                                yT[po : po + HS, ch, qsl], yp[0:HS, :],
                                zb, ALU.mult,
                            )

                # ---- attn proj + residual ----
                pbr = lnp.tile([1, E], BF16, name=f"pbr{l}", tag="pbr")
                nc.sync.dma_start(pbr, pbr_d[l : l + 1, :])
                pw_sb = []
                for c in range(EC):
                    pwc = pwp.tile([P, E], BF16, name=f"pw{l}_{c}", tag="pw")
                    nc.sync.dma_start(pwc, pw_d[l, c * P : (c + 1) * P, :])
                    pw_sb.append(pwc)
                for co in range(EC):
                    ps = ps2.tile([P, T], F32, name=f"pj{l}_{co}", tag="ps2")
                    osl = slice(co * P, (co + 1) * P)
                    for c in range(EC):
                        last = no_bias and c == EC - 1
                        for qh in range(QH):
                            sl = slice(qh * 512, (qh + 1) * 512)
                            nc.tensor.matmul(
                                ps[:, sl], pw_sb[c][:, osl], yT[:, c, sl],
                                start=(c == 0), stop=last,
                            )
                    for qh in range(QH):
                        sl = slice(qh * 512, (qh + 1) * 512)
                        if not no_bias:
                            nc.tensor.matmul(ps[:, sl], pbr[:, osl], ones_512,
                                             start=False, stop=True)
                        nc.vector.tensor_tensor(xTh[qh][:, co, :],
                                                xTh[qh][:, co, :], ps[:, sl],
                                                ALU.add)

                # ---- MLP: fc -> gelu -> fcp interleaved per hidden strip ----
                fcb = lnp.tile([P, 24], F32, name=f"fcb{l}", tag="fcb")
                nc.sync.dma_start(fcb, fcb_d[l])
                fpbr = lnp.tile([1, E], BF16, name=f"fpbr{l}", tag="fpbr")
                nc.sync.dma_start(fpbr, fpbr_d[l : l + 1, :])
                ln2 = layer_norm(None, None, f"b{l}")

                for qh in range(QH):
                    sl = slice(qh * 512, (qh + 1) * 512)
                    pss = [
                        ps2.tile([P, 1024], F32, name=f"fp{l}_{qh}_{cp}",
                                 tag="ps2")
                        for cp in range(3)
                    ]
                    for grp in range(4):
                        fw_sb = []
                        for c in range(EC):
                            fwc = fwp.tile([P, E], BF16,
                                           name=f"fw{l}_{qh}_{grp}_{c}", tag="fw")
                            nc.sync.dma_start(
                                fwc, fw_d[l, c * P : (c + 1) * P,
                                          grp * E : (grp + 1) * E]
                            )
                            fw_sb.append(fwc)
                        for t in range(EC):
                            hc = grp * EC + t
                            hps = ps1.tile([P, 512], F32, name=f"fc{l}_{qh}_{hc}",
                                           tag="ps1")
                            for c in range(EC):
                                nc.tensor.matmul(
                                    hps, fw_sb[c][:, t * P : (t + 1) * P],
                                    ln2[:, c, sl],
                                    start=(c == 0), stop=(c == EC - 1),
                                )
                            ht = htp.tile([P, 512], BF16, name=f"hT{l}_{qh}_{hc}",
                                          tag="hT")
                            nc.scalar.activation(ht, hps, AF.Gelu,
                                                 bias=fcb[:, hc : hc + 1])
                            fpc = fpwp.tile([P, E], BF16,
                                            name=f"fpw{l}_{qh}_{hc}", tag="fpw")
                            nc.sync.dma_start(fpc,
                                              fpw_d[l, hc * P : (hc + 1) * P, :])
                            for co in range(EC):
                                nc.tensor.matmul(
                                    pss[co // 2][:, (co % 2) * 512
                                                 : (co % 2 + 1) * 512],
                                    fpc[:, co * P : (co + 1) * P], ht,
                                    start=(hc == 0),
                                    stop=(no_bias and hc == 23),
                                )
                    for co in range(EC):
                        psl = slice((co % 2) * 512, (co % 2 + 1) * 512)
                        if not no_bias:
                            nc.tensor.matmul(
                                pss[co // 2][:, psl],
                                fpbr[:, co * P : (co + 1) * P],
                                ones_512, start=False, stop=True,
                            )
                        nc.vector.tensor_tensor(
                            xTh[qh][:, co, :], xTh[qh][:, co, :],
                            pss[co // 2][:, psl],
                            ALU.add,
                        )

            # ---- final LN + lm_head (own vocab shard, all T tokens) ----
            lnf = layer_norm(lnfw, lnfb, "f")
            for vc in range(NVC):
                w = min(512, VS - vc * 512)
                wts = []
                for c in range(EC):
                    wtc = wtp.tile([P, 512], BF16, name=f"wt{vc}_{c}", tag="wt")
                    nc.sync.dma_start(
                        wtc[:, :w],
                        wteT_d[c * P : (c + 1) * P, vc * 512 : vc * 512 + w],
                    )
                    wts.append(wtc)
                for tb in range(TB):
                    if tb % 2 == 0:
                        lps = ps2.tile([P, 1024], F32, name=f"lm{vc}_{tb}",
                                       tag="ps2")
                    psl = slice((tb % 2) * 512, (tb % 2) * 512 + w)
                    for c in range(EC):
                        nc.tensor.matmul(
                            lps[:, psl],
                            lnf[:, c, tb * P : (tb + 1) * P],
                            wts[c][:, :w],
                            start=(c == 0), stop=(c == EC - 1),
                        )
                    o = lop.tile([P, 512], F32, name=f"lo{vc}_{tb}", tag="lo")
                    if tb % 2 == 0:
                        nc.scalar.copy(o[:, :w], lps[:, psl])
                    else:
                        nc.vector.tensor_copy(o[:, :w], lps[:, psl])
                    nc.sync.dma_start(
                        out_d[tb * P : (tb + 1) * P, vc * 512 : vc * 512 + w],
                        o[:, :w],
                    )

    nc.compile()
    return nc


_CACHE = {}


def _get_nc(L, VS, no_bias=False):
    key = (L, VS, no_bias)
    if key not in _CACHE:
        _CACHE[key] = _build(L, VS, no_bias=no_bias)
    return _CACHE[key]


def _bf(a):
    return np.ascontiguousarray(a.astype(_nbf))


def _pp(a, cols):
    """[L?, n*128] feature vector -> per-partition layout [..., 128, n]."""
    a = np.asarray(a, np.float32)
    shp = a.shape[:-1]
    n = a.shape[-1] // P
    return np.ascontiguousarray(a.reshape(*shp, n, P).swapaxes(-1, -2))


def _prepare(inputs, L, VS):
    """Host prep: embedding, weight cast/fold/transpose, per-core in_maps."""
    idx = np.asarray(inputs["idx"])
    wte = np.asarray(inputs["wte"], np.float32)
    wpe = np.asarray(inputs["wpe"], np.float32)

    x0 = wte[idx] + wpe[None, :T]  # [B, T, E] f32

    ln1_w = np.asarray(inputs["ln1_w"], np.float32)[:L]
    ln1_b = np.asarray(inputs["ln1_b"], np.float32)[:L]
    ln2_w = np.asarray(inputs["ln2_w"], np.float32)[:L]
    ln2_b = np.asarray(inputs["ln2_b"], np.float32)[:L]

    attn_w = np.asarray(inputs["attn_w"], np.float32)[:L].copy()
    attn_b = np.asarray(inputs["attn_b"], np.float32)[:L].copy()
    scale = 1.0 / np.sqrt(HS)
    attn_w[:, :, :E] *= scale
    attn_b[:, :E] *= scale
    # fold ln1 affine: ln(x)@W + b == n(x)@(w*W) + (b + ln_b@W)
    attn_b = attn_b + np.einsum("le,leo->lo", ln1_b, attn_w)
    attn_w = attn_w * ln1_w[:, :, None]

    fc_w = np.asarray(inputs["fc_w"], np.float32)[:L].copy()
    fc_b = np.asarray(inputs["fc_b"], np.float32)[:L].copy()
    fc_b = fc_b + np.einsum("le,leo->lo", ln2_b, fc_w)
    fc_w = fc_w * ln2_w[:, :, None]

    aw = _bf(attn_w)
    pw = _bf(np.asarray(inputs["proj_w"], np.float32)[:L])
    fw = _bf(fc_w)
    fpw = _bf(np.asarray(inputs["fcp_w"], np.float32)[:L])

    lnfp = np.stack(
        [_pp(np.asarray(inputs["lnf_w"], np.float32), EC),
         _pp(np.asarray(inputs["lnf_b"], np.float32), EC)], axis=0
    )
    qkb = _pp(attn_b[:, : 2 * E], 12)
    vbr = _bf(attn_b[:, 2 * E :])
    pbr = _bf(np.asarray(inputs["proj_b"], np.float32)[:L])
    fcb = _pp(fc_b, 24)
    fpbr = _bf(np.asarray(inputs["fcp_b"], np.float32)[:L])

    # wteT padded + per-core vocab shards (4 shards across each batch group)
    wteT = np.zeros((E, 4 * VS), _nbf)
    nv = min(V, 4 * VS)
    wteT[:, :nv] = _bf(wte.T[:, :nv])

    # multiplicative causal mask for the diagonal 128x128 score block
    kpos = np.arange(P)
    msk = (kpos[:, None] <= kpos[None, :]).astype(_nbf)

    in_maps = []
    for c in range(NCORE):
        b = c // 4
        sh = c % 4
        x0T = np.ascontiguousarray(x0[b].T)  # [768, 1024]
        in_maps.append(
            {
                "x0T": x0T,
                "lnfp": lnfp,
                "qkb": qkb, "vbr": vbr, "pbr": pbr, "fcb": fcb, "fpbr": fpbr,
                "aw": aw, "pw": pw, "fw": fw, "fpw": fpw,
                "wteT": np.ascontiguousarray(wteT[:, sh * VS : (sh + 1) * VS]),
                "msk": msk,
            }
        )
    return in_maps


def _run(inputs, L, VS, trace=False):
    in_maps = _prepare(inputs, L, VS)
    m0 = in_maps[0]
    no_bias = bool(
        not np.any(m0["vbr"]) and not np.any(m0["pbr"]) and not np.any(m0["fpbr"])
    )
    nc = _get_nc(L, VS, no_bias=no_bias)
    res = run_bass_kernel_spmd(
        nc, in_maps, core_ids=list(range(NCORE)), trace=trace
    )
    # core c holds batch c//4, vocab shard c%4: concat shards per batch
    outs = [res.results[c]["out"] for c in range(NCORE)]
    logits = np.stack(
        [np.concatenate(outs[4 * b : 4 * b + 4], axis=1)[:, :V] for b in range(B)]
    )
    return np.ascontiguousarray(logits), res


def kernel(**inputs) -> np.ndarray:
    trace = bool(os.environ.get("_KERNEL_TRACE"))
    logits, _ = _run(inputs, L_FULL, VSH, trace=trace)
    return logits


if __name__ == "__main__":
    pass


# revision 31
# speedup vs baseline: 1.2721x; 1.0114x over previous
"""GPT-2 small forward pass on 8 Trainium2 NeuronCores (Bass/Tile).

ZERO-COLLECTIVE design: each 4-core group redundantly computes the full
12-layer trunk for its batch (cores 0-3 batch 0, cores 4-7 batch 1,
T=1024 tokens each), then each core computes the lm_head for its own
quarter vocab shard (4 x 12565 >= 50257). No cross-core communication at
all -- the 4x redundant trunk compute runs on cores that would otherwise
idle, and removes the per-layer K/V AllGathers (the dominant real-HW cost
of the previous sequence-parallel design).

Layout: activations feature-major xT [768, 1024] on-chip; scores computed
transposed (K stationary); PV uses token-major V with an appended ones
column so softmax denominators fall out of the same matmul. Causality on
the (otherwise idle) GpSimd engine AFTER exp: fully-invalid column spans
are memset to zero and the diagonal 128-wide span is multiplied by one
shared triangular 0/1 bf16 mask; fully-masked score blocks are skipped.
Attention runs per head-pair (q/k strips computed just-in-time);
fc->gelu->fcp interleave per hidden strip so the MLP hidden never
materializes fully.

Precision: bf16 matmul inputs, f32 PSUM/residual/LN math. Softmax without
max subtraction (scores bounded ~[-2.3, 2.7] for this model's init scale).
"""

import os
import sys

import numpy as np
import ml_dtypes

sys.path.insert(0, "/opt/trn_rl_repo")

import concourse.mybir as mybir  # noqa: E402
import concourse.tile as tile  # noqa: E402
from concourse import bacc  # noqa: E402
from concourse.bass_utils import run_bass_kernel_spmd  # noqa: E402

BF16 = mybir.dt.bfloat16
F32 = mybir.dt.float32
AF = mybir.ActivationFunctionType
ALU = mybir.AluOpType

P = 128
E = 768
EC = E // P  # 6
H = 12
HS = 64
B = 2
T = 1024
TB = T // P  # 8 token blocks
QH = 2  # 512-wide column halves
L_FULL = 12
V = 50257
NCORE = 8
VSH = 12565  # per-core vocab shard (4*12565 = 50260 >= V)
NVC = (VSH + 511) // 512  # 25 vocab chunks
EPS = 1e-5

_nbf = ml_dtypes.bfloat16


def _build(L, VS, no_bias=False):
    from contextlib import ExitStack

    nc = bacc.Bacc("TRN2", target_bir_lowering=False, debug=False, num_devices=NCORE)

    # ---- DRAM I/O (per core; batch/shard selection is host-side) ----
    x0T_d = nc.dram_tensor("x0T", [E, T], F32, kind="ExternalInput").ap()
    lnf_d = nc.dram_tensor("lnfp", [2, P, EC], F32, kind="ExternalInput").ap()
    qkb_d = nc.dram_tensor("qkb", [L, P, 12], F32, kind="ExternalInput").ap()
    vbr_d = nc.dram_tensor("vbr", [L, E], BF16, kind="ExternalInput").ap()
    pbr_d = nc.dram_tensor("pbr", [L, E], BF16, kind="ExternalInput").ap()
    fcb_d = nc.dram_tensor("fcb", [L, P, 24], F32, kind="ExternalInput").ap()
    fpbr_d = nc.dram_tensor("fpbr", [L, E], BF16, kind="ExternalInput").ap()
    aw_d = nc.dram_tensor("aw", [L, E, 3 * E], BF16, kind="ExternalInput").ap()
    pw_d = nc.dram_tensor("pw", [L, E, E], BF16, kind="ExternalInput").ap()
    fw_d = nc.dram_tensor("fw", [L, E, 4 * E], BF16, kind="ExternalInput").ap()
    fpw_d = nc.dram_tensor("fpw", [L, 4 * E, E], BF16, kind="ExternalInput").ap()
    wteT_d = nc.dram_tensor("wteT", [E, VS], BF16, kind="ExternalInput").ap()
    msk_d = nc.dram_tensor("msk", [P, P], BF16, kind="ExternalInput").ap()
    out_d = nc.dram_tensor("out", [T, VS], F32, kind="ExternalOutput").ap()

    with tile.TileContext(nc) as tc:
        with ExitStack() as stack:
            pool_specs = [
                ("persist", 1, None),  # constants + masks
                ("resid", 1, None),    # xT f32
                ("lnp", 2, None),      # per-layer small params
                ("lnstage", 1, None),  # ln1/ln2/lnf [P,EC,T] bf16
                ("xbp", 3, None),      # xb/sq bf16 copies for LN stats
                ("smallp", 1, None),   # [1,T] stat rows
                ("awqk", 2, None),     # per-head-pair q/k weight strips
                ("awv", 7, None),      # v weight strips [P,768]
                ("pwp", 7, None),      # proj weight strips
                ("fwp", 8, None),      # fc weight strips
                ("fpwp", 4, None),     # fcp weight strips
                ("qkc", 3, None),      # per-head-pair qT/kT strips [P,T]
                ("vap", 1, None),      # va token-major tiles
                ("ep", 2, None),       # exp'd score tiles
                ("ytp", 1, None),      # yT
                ("htp", 4, None),      # transient hidden strips [P,512]
                ("tmpp", 2, None),     # LN tmp + zb
                ("wtp", 8, None),      # lm_head weight chunks
                ("lop", 3, None),      # lm_head output staging
                ("ps2", 3, "PSUM"),    # [P,1024] 2-bank psums
                ("ps1", 2, "PSUM"),    # [P,512] 1-bank psums
            ]
            pools = {}
            for pname, bufs, space in pool_specs:
                kw = {"name": pname, "bufs": bufs}
                if space:
                    kw["space"] = space
                pools[pname] = stack.enter_context(tc.tile_pool(**kw))
            (persist, resid, lnp, lnstage, xbp, smallp, awqk, awv, pwp, fwp,
             fpwp, qkc, vap, ep, ytp, htp, tmpp, wtp, lop, ps2, ps1) = (
                pools[n] for n, _, _ in pool_specs
            )

            # ---- persistent constants ----
            ones_col_f = persist.tile([P, 1], F32, name="ones_col_f")
            nc.vector.memset(ones_col_f, 1.0)
            ones_col = persist.tile([P, 1], BF16, name="ones_col")
            nc.vector.tensor_copy(ones_col, ones_col_f)
            ones_row_f = persist.tile([1, P], F32, name="ones_row_f")
            nc.vector.memset(ones_row_f, 1.0)
            ones_row = persist.tile([1, P], BF16, name="ones_row")
            nc.vector.tensor_copy(ones_row, ones_row_f)
            ones_512f = persist.tile([1, 512], F32, name="ones_512f")
            nc.vector.memset(ones_512f, 1.0)
            ones_512 = persist.tile([1, 512], BF16, name="ones_512")
            nc.vector.tensor_copy(ones_512, ones_512f)
            eps_sb = persist.tile([P, 1], F32, name="eps_sb")
            nc.vector.memset(eps_sb, EPS)
            mask_sb = persist.tile([P, P], BF16, name="mask_sb")
            nc.sync.dma_start(mask_sb, msk_d)
            lnfw = persist.tile([P, EC], F32, name="lnfw")
            lnfb = persist.tile([P, EC], F32, name="lnfb")
            nc.sync.dma_start(lnfw, lnf_d[0])
            nc.sync.dma_start(lnfb, lnf_d[1])

            # ---- residual (two half-tiles so qh-half deps stay independent) ----
            xTh = [resid.tile([P, EC, 512], F32, name=f"xT{qh}") for qh in range(QH)]
            for c in range(EC):
                for qh in range(QH):
                    nc.sync.dma_start(
                        xTh[qh][:, c, :],
                        x0T_d[c * P : (c + 1) * P, qh * 512 : (qh + 1) * 512],
                    )

            def layer_norm(w_sb, b_sb, tagp):
                """xT (f32, resid) -> new bf16 tile [P, EC, T].
                When w_sb is None the affine transform is folded into the
                consumer's weights/biases host-side and the normalized value
                is written directly by the second DVE op.

                Fully split per 512-column half so each half's chain only
                depends on that half of the residual (overlaps the previous
                phase's second half).
                """
                s1 = ps2.tile([1, T], F32, name=f"s1{tagp}", tag="ps2")
                s2 = ps2.tile([1, T], F32, name=f"s2{tagp}", tag="ps2")
                for c in range(EC):
                    for qh in range(QH):
                        sl = slice(qh * 512, (qh + 1) * 512)
                        xb = xbp.tile([P, 512], BF16, name=f"xb{tagp}", tag="xb")
                        sq = xbp.tile([P, 512], BF16, name=f"sq{tagp}", tag="sq")
                        nc.vector.tensor_copy(xb, xTh[qh][:, c, :])
                        nc.scalar.activation(sq, xTh[qh][:, c, :], AF.Square)
                        nc.tensor.matmul(
                            s1[:, sl], ones_col, xb,
                            start=(c == 0), stop=(c == EC - 1),
                        )
                        nc.tensor.matmul(
                            s2[:, sl], ones_col, sq,
                            start=(c == 0), stop=(c == EC - 1),
                        )
                rstd_b = smallp.tile([1, T], BF16, name=f"rstdb{tagp}", tag="smr")
                mr_b = smallp.tile([1, T], BF16, name=f"mrb{tagp}", tag="smb")
                a_sb = tmpp.tile([P, T], F32, name=f"asb{tagp}", tag="asb")
                c_sb = tmpp.tile([P, T], F32, name=f"csb{tagp}", tag="csb")
                for qh in range(QH):
                    sl = slice(qh * 512, (qh + 1) * 512)
                    var = smallp.tile([1, 512], F32, name=f"var{tagp}_{qh}",
                                      tag="smv")
                    mean = smallp.tile([1, 512], F32, name=f"mean{tagp}_{qh}",
                                       tag="smm")
                    nc.vector.tensor_scalar_mul(mean, s1[:, sl], 1.0 / E)
                    nc.vector.tensor_tensor(var, mean, mean, ALU.mult)
                    nc.vector.scalar_tensor_tensor(
                        var, s2[:, sl], 1.0 / E, var, ALU.mult, ALU.subtract,
                    )
                    nc.scalar.activation(var, var, AF.Sqrt, bias=eps_sb[0:1])
                    with nc.allow_low_precision(
                        reason="rstd feeds a bf16 broadcast matmul"
                    ):
                        nc.vector.reciprocal(rstd_b[:, sl], var)
                        # mr = (s1/E) * rstd
                        nc.vector.scalar_tensor_tensor(
                            mr_b[:, sl], s1[:, sl], 1.0 / E, rstd_b[:, sl],
                            ALU.mult, ALU.mult,
                        )
                    bc_ps = ps2.tile([P, T], F32, name=f"bc{tagp}_{qh}", tag="ps2")
                    nc.tensor.matmul(bc_ps[:, 0:512], ones_row, rstd_b[:, sl],
                                     start=True, stop=True)
                    nc.tensor.matmul(bc_ps[:, 512:1024], ones_row, mr_b[:, sl],
                                     start=True, stop=True)
                    nc.scalar.copy(a_sb[:, sl], bc_ps[:, 0:512])
                    nc.vector.tensor_copy(c_sb[:, sl], bc_ps[:, 512:1024])
                ln = lnstage.tile([P, EC, T], BF16, name=f"ln{tagp}", tag="ln")
                for c in range(EC):
                    for qh in range(QH):
                        sl = slice(qh * 512, (qh + 1) * 512)
                        eng = nc.vector if c % 2 == 0 else nc.gpsimd
                        tmp = tmpp.tile([P, 512], F32, name=f"lt{tagp}", tag="lntmp")
                        eng.tensor_tensor(tmp, xTh[qh][:, c, :], a_sb[:, sl],
                                          ALU.mult)
                        if w_sb is None:
                            with nc.allow_low_precision(
                                reason="normalized activation cast to bf16 for matmul"
                            ):
                                eng.tensor_tensor(ln[:, c, sl], tmp,
                                                  c_sb[:, sl], ALU.subtract)
                        else:
                            eng.tensor_tensor(tmp, tmp, c_sb[:, sl],
                                              ALU.subtract)
                            nc.scalar.activation(
                                ln[:, c, sl], tmp, AF.Identity,
                                bias=b_sb[:, c : c + 1], scale=w_sb[:, c : c + 1],
                            )
                return ln

            for l in range(L):
                # ---- layer params ----
                qkb = lnp.tile([P, 12], F32, name=f"qkb{l}", tag="qkb")
                nc.sync.dma_start(qkb, qkb_d[l])
                vbr = lnp.tile([1, E], BF16, name=f"vbr{l}", tag="vbr")
                nc.sync.dma_start(vbr, vbr_d[l : l + 1, :])

                ln1 = layer_norm(None, None, f"a{l}")

                # ---- v token-major va[tb]: [128 tok, H, HS+1] bf16 ----
                awv_sb = []
                for c in range(EC):
                    avc = awv.tile([P, E], BF16, name=f"awv{l}_{c}", tag="awv")
                    nc.sync.dma_start(avc, aw_d[l, c * P : (c + 1) * P, 2 * E :])
                    awv_sb.append(avc)
                va_t = [
                    vap.tile([P, H, HS + 1], BF16, name=f"va{l}_{tb}", tag=f"va{tb}")
                    for tb in range(TB)
                ]
                for tb in range(TB):
                    nc.gpsimd.memset(va_t[tb][:, :, HS : HS + 1], 1.0)
                    vps = ps2.tile([P, 1024], F32, name=f"v{l}_{tb}", tag="ps2")
                    tsl = slice(tb * P, (tb + 1) * P)
                    for c in range(EC):
                        last = no_bias and c == EC - 1
                        nc.tensor.matmul(
                            vps[:, 0:512], ln1[:, c, tsl], awv_sb[c][:, 0:512],
                            start=(c == 0), stop=last,
                        )
                        nc.tensor.matmul(
                            vps[:, 512:768], ln1[:, c, tsl], awv_sb[c][:, 512:768],
                            start=(c == 0), stop=last,
                        )
                    if not no_bias:
                        nc.tensor.matmul(vps[:, 0:512], ones_row, vbr[:, 0:512],
                                         start=False, stop=True)
                        nc.tensor.matmul(vps[:, 512:768], ones_row,
                                         vbr[:, 512:768], start=False, stop=True)
                    nc.vector.tensor_copy(
                        va_t[tb][:, :, 0:HS],
                        vps[:, 0:768].rearrange("p (h d) -> p h d", d=HS),
                    )

                # ---- attention per head-pair ch: q/k strips -> per-head
                # scores -> exp -> mask -> PV -> /Z ----
                yT = ytp.tile([P, EC, T], BF16, name=f"yT{l}", tag="yT")
                blocks = {0: range(4), 1: range(8)}
                for ch in range(EC):
                    # q/k weight strips for this head pair: [P, EC, P] each
                    wq = awqk.tile([P, EC, P], BF16, name=f"wq{l}_{ch}", tag="wq")
                    wk = awqk.tile([P, EC, P], BF16, name=f"wk{l}_{ch}", tag="wk")
                    nc.sync.dma_start(
                        wq, aw_d[l][:, ch * P : (ch + 1) * P]
                        .rearrange("(c p) n -> p c n", p=P)
                    )
                    nc.sync.dma_start(
                        wk, aw_d[l][:, E + ch * P : E + (ch + 1) * P]
                        .rearrange("(c p) n -> p c n", p=P)
                    )
                    qTc = qkc.tile([P, T], BF16, name=f"qT{l}_{ch}", tag="qTc")
                    kTc = qkc.tile([P, T], BF16, name=f"kT{l}_{ch}", tag="kTc")
                    for wgt, dst, bofs in ((wq, qTc, 0), (wk, kTc, EC)):
                        pq = ps2.tile([P, T], F32, name=f"qk{l}_{ch}_{bofs}",
                                      tag="ps2")
                        for c in range(EC):
                            for qh in range(QH):
                                sl = slice(qh * 512, (qh + 1) * 512)
                                nc.tensor.matmul(
                                    pq[:, sl], wgt[:, c, :], ln1[:, c, sl],
                                    start=(c == 0), stop=(c == EC - 1),
                                )
                        for qh in range(QH):
                            sl = slice(qh * 512, (qh + 1) * 512)
                            with nc.allow_low_precision(
                                reason="q/k cast to bf16 for score matmuls"
                            ):
                                nc.vector.tensor_scalar_add(
                                    dst[:, sl], pq[:, sl],
                                    qkb[:, bofs + ch : bofs + ch + 1],
                                )
                    for hh in range(2):
                        h = 2 * ch + hh
                        po = hh * HS
                        Es = {}  # (qh, j) -> (tile, col_slice)
                        for qh in (1, 0):
                            qsl = slice(qh * 512, (qh + 1) * 512)
                            js = list(blocks[qh])
                            for jp in range(0, len(js), 2):
                                scp = ps2.tile([P, 1024], F32,
                                               name=f"sc{l}_{h}_{qh}_{jp}",
                                               tag="ps2")
                                for k2, j in enumerate(js[jp : jp + 2]):
                                    nc.tensor.matmul(
                                        scp[:, k2 * 512 : (k2 + 1) * 512],
                                        kTc[po : po + HS, j * P : (j + 1) * P],
                                        qTc[po : po + HS, qsl],
                                        start=True, stop=True,
                                    )
                                et = ep.tile([P, 1024], BF16,
                                             name=f"E{l}_{h}_{qh}_{jp}",
                                             tag=f"E{jp}")
                                nc.scalar.activation(et, scp, AF.Exp)
                                for k2, j in enumerate(js[jp : jp + 2]):
                                    c0 = k2 * 512
                                    csl = slice(c0, c0 + 512)
                                    r = j * P - qh * 512
                                    if r >= 0:  # partially masked block
                                        if r > 0:
                                            nc.gpsimd.memset(
                                                et[:, c0 : c0 + r], 0.0)
                                        nc.vector.tensor_tensor(
                                            et[:, c0 + r : c0 + r + P],
                                            et[:, c0 + r : c0 + r + P],
                                            mask_sb, ALU.mult,
                                        )
                                    Es[(qh, j)] = (et, csl)
                        zinv_b = smallp.tile([1, T], BF16, name=f"zib{l}_{h}",
                                             tag="zib")
                        for qh in (1, 0):
                            js = list(blocks[qh])
                            yp = ps1.tile([P, 512], F32, name=f"y{l}_{h}_{qh}",
                                          tag="ps1")
                            for ji, j in enumerate(js):
                                et, csl = Es[(qh, j)]
                                nc.tensor.matmul(
                                    yp[0 : HS + 1, :], va_t[j][:, h, :],
                                    et[:, csl],
                                    start=(ji == 0), stop=(ji == len(js) - 1),
                                )
                            qsl = slice(qh * 512, (qh + 1) * 512)
                            with nc.allow_low_precision(
                                reason="1/Z in bf16 feeds a bf16 matmul anyway"
                            ):
                                nc.vector.reciprocal(zinv_b[:, qsl],
                                                     yp[HS : HS + 1, :])
                            zb_ps = ps1.tile([P, 512], F32, name=f"zp{l}_{h}_{qh}",
                                             tag="ps1")
                            nc.tensor.matmul(zb_ps[0:HS, :], ones_row[:, 0:HS],
                                             zinv_b[:, qsl], start=True,
                                             stop=True)
                            zb = tmpp.tile([HS, 512], F32, name=f"zb{l}_{h}",
                                           tag="zb")
                            nc.vector.tensor_copy(zb, zb_ps[0:HS, :])
                            nc.vector.tensor_tensor(
                                yT[po : po + HS, ch, qsl], yp[0:HS, :],
                                zb, ALU.mult,
                            )

                # ---- attn proj + residual ----
                pbr = lnp.tile([1, E], BF16, name=f"pbr{l}", tag="pbr")
                nc.sync.dma_start(pbr, pbr_d[l : l + 1, :])
                pw_sb = []
                for c in range(EC):
                    pwc = pwp.tile([P, E], BF16, name=f"pw{l}_{c}", tag="pw")
                    nc.sync.dma_start(pwc, pw_d[l, c * P : (c + 1) * P, :])
                    pw_sb.append(pwc)
                for co in range(EC):
                    ps = ps2.tile([P, T], F32, name=f"pj{l}_{co}", tag="ps2")
                    osl = slice(co * P, (co + 1) * P)
                    for c in range(EC):
                        last = no_bias and c == EC - 1
                        for qh in range(QH):
                            sl = slice(qh * 512, (qh + 1) * 512)
                            nc.tensor.matmul(
                                ps[:, sl], pw_sb[c][:, osl], yT[:, c, sl],
                                start=(c == 0), stop=last,
                            )
                    for qh in range(QH):
                        sl = slice(qh * 512, (qh + 1) * 512)
                        if not no_bias:
                            nc.tensor.matmul(ps[:, sl], pbr[:, osl], ones_512,
                                             start=False, stop=True)
                        nc.vector.tensor_tensor(xTh[qh][:, co, :],
                                                xTh[qh][:, co, :], ps[:, sl],
                                                ALU.add)

                # ---- MLP: fc -> gelu -> fcp interleaved per hidden strip ----
                fcb = lnp.tile([P, 24], F32, name=f"fcb{l}", tag="fcb")
                nc.sync.dma_start(fcb, fcb_d[l])
                fpbr = lnp.tile([1, E], BF16, name=f"fpbr{l}", tag="fpbr")
                nc.sync.dma_start(fpbr, fpbr_d[l : l + 1, :])
                ln2 = layer_norm(None, None, f"b{l}")

                for qh in range(QH):
                    sl = slice(qh * 512, (qh + 1) * 512)
                    pss = [
                        ps2.tile([P, 1024], F32, name=f"fp{l}_{qh}_{cp}",
                                 tag="ps2")
                        for cp in range(3)
                    ]
                    for grp in range(4):
                        fw_sb = []
                        for c in range(EC):
                            fwc = fwp.tile([P, E], BF16,
                                           name=f"fw{l}_{qh}_{grp}_{c}", tag="fw")
                            nc.sync.dma_start(
                                fwc, fw_d[l, c * P : (c + 1) * P,
                                          grp * E : (grp + 1) * E]
                            )
                            fw_sb.append(fwc)
                        for t in range(EC):
                            hc = grp * EC + t
                            hps = ps1.tile([P, 512], F32, name=f"fc{l}_{qh}_{hc}",
                                           tag="ps1")
                            for c in range(EC):
                                nc.tensor.matmul(
                                    hps, fw_sb[c][:, t * P : (t + 1) * P],
                                    ln2[:, c, sl],
                                    start=(c == 0), stop=(c == EC - 1),
                                )
                            ht = htp.tile([P, 512], BF16, name=f"hT{l}_{qh}_{hc}",
                                          tag="hT")
                            nc.scalar.activation(ht, hps, AF.Gelu,
                                                 bias=fcb[:, hc : hc + 1])
                            fpc = fpwp.tile([P, E], BF16,
                                            name=f"fpw{l}_{qh}_{hc}", tag="fpw")
                            nc.sync.dma_start(fpc,
                                              fpw_d[l, hc * P : (hc + 1) * P, :])
                            for co in range(EC):
                                nc.tensor.matmul(
                                    pss[co // 2][:, (co % 2) * 512
                                                 : (co % 2 + 1) * 512],
                                    fpc[:, co * P : (co + 1) * P], ht,
                                    start=(hc == 0),
                                    stop=(no_bias and hc == 23),
                                )
                    for co in range(EC):
                        psl = slice((co % 2) * 512, (co % 2 + 1) * 512)
                        if not no_bias:
                            nc.tensor.matmul(
                                pss[co // 2][:, psl],
                                fpbr[:, co * P : (co + 1) * P],
                                ones_512, start=False, stop=True,
                            )
                        nc.vector.tensor_tensor(
                            xTh[qh][:, co, :], xTh[qh][:, co, :],
                            pss[co // 2][:, psl],
                            ALU.add,
                        )

            # ---- final LN + lm_head (own vocab shard, all T tokens) ----
            lnf = layer_norm(lnfw, lnfb, "f")
            for vc in range(NVC):
                w = min(512, VS - vc * 512)
                wts = []
                for c in range(EC):
                    wtc = wtp.tile([P, 512], BF16, name=f"wt{vc}_{c}", tag="wt")
                    nc.sync.dma_start(
                        wtc[:, :w],
                        wteT_d[c * P : (c + 1) * P, vc * 512 : vc * 512 + w],
                    )
                    wts.append(wtc)
                for tb in range(TB):
                    if tb % 2 == 0:
                        lps = ps2.tile([P, 1024], F32, name=f"lm{vc}_{tb}",
                                       tag="ps2")
                    psl = slice((tb % 2) * 512, (tb % 2) * 512 + w)
                    for c in range(EC):
                        nc.tensor.matmul(
                            lps[:, psl],
                            lnf[:, c, tb * P : (tb + 1) * P],
                            wts[c][:, :w],
                            start=(c == 0), stop=(c == EC - 1),
                        )
                    o = lop.tile([P, 512], F32, name=f"lo{vc}_{tb}", tag="lo")
                    if tb % 2 == 0:
                        nc.scalar.copy(o[:, :w], lps[:, psl])
                    else:
                        nc.vector.tensor_copy(o[:, :w], lps[:, psl])
                    nc.sync.dma_start(
                        out_d[tb * P : (tb + 1) * P, vc * 512 : vc * 512 + w],
                        o[:, :w],
                    )

    nc.compile()
    return nc


_CACHE = {}


def _get_nc(L, VS, no_bias=False):
    key = (L, VS, no_bias)
    if key not in _CACHE:
        _CACHE[key] = _build(L, VS, no_bias=no_bias)
    return _CACHE[key]


def _bf(a):
    return np.ascontiguousarray(a.astype(_nbf))


def _pp(a, cols):
    """[L?, n*128] feature vector -> per-partition layout [..., 128, n]."""
    a = np.asarray(a, np.float32)
    shp = a.shape[:-1]
    n = a.shape[-1] // P
    return np.ascontiguousarray(a.reshape(*shp, n, P).swapaxes(-1, -2))


def _prepare(inputs, L, VS):
    """Host prep: embedding, weight cast/fold/transpose, per-core in_maps."""
    idx = np.asarray(inputs["idx"])
    wte = np.asarray(inputs["wte"], np.float32)
    wpe = np.asarray(inputs["wpe"], np.float32)

    x0 = wte[idx] + wpe[None, :T]  # [B, T, E] f32

    ln1_w = np.asarray(inputs["ln1_w"], np.float32)[:L]
    ln1_b = np.asarray(inputs["ln1_b"], np.float32)[:L]
    ln2_w = np.asarray(inputs["ln2_w"], np.float32)[:L]
    ln2_b = np.asarray(inputs["ln2_b"], np.float32)[:L]

    attn_w = np.asarray(inputs["attn_w"], np.float32)[:L].copy()
    attn_b = np.asarray(inputs["attn_b"], np.float32)[:L].copy()
    scale = 1.0 / np.sqrt(HS)
    attn_w[:, :, :E] *= scale
    attn_b[:, :E] *= scale
    # fold ln1 affine: ln(x)@W + b == n(x)@(w*W) + (b + ln_b@W)
    attn_b = attn_b + np.einsum("le,leo->lo", ln1_b, attn_w)
    attn_w = attn_w * ln1_w[:, :, None]

    fc_w = np.asarray(inputs["fc_w"], np.float32)[:L].copy()
    fc_b = np.asarray(inputs["fc_b"], np.float32)[:L].copy()
    fc_b = fc_b + np.einsum("le,leo->lo", ln2_b, fc_w)
    fc_w = fc_w * ln2_w[:, :, None]

    aw = _bf(attn_w)
    pw = _bf(np.asarray(inputs["proj_w"], np.float32)[:L])
    fw = _bf(fc_w)
    fpw = _bf(np.asarray(inputs["fcp_w"], np.float32)[:L])

    lnfp = np.stack(
        [_pp(np.asarray(inputs["lnf_w"], np.float32), EC),
         _pp(np.asarray(inputs["lnf_b"], np.float32), EC)], axis=0
    )
    qkb = _pp(attn_b[:, : 2 * E], 12)
    vbr = _bf(attn_b[:, 2 * E :])
    pbr = _bf(np.asarray(inputs["proj_b"], np.float32)[:L])
    fcb = _pp(fc_b, 24)
    fpbr = _bf(np.asarray(inputs["fcp_b"], np.float32)[:L])

    # wteT padded + per-core vocab shards (4 shards across each batch group)
    wteT = np.zeros((E, 4 * VS), _nbf)
    nv = min(V, 4 * VS)
    wteT[:, :nv] = _bf(wte.T[:, :nv])

    # multiplicative causal mask for the diagonal 128x128 score block
    kpos = np.arange(P)
    msk = (kpos[:, None] <= kpos[None, :]).astype(_nbf)

    in_maps = []
    for c in range(NCORE):
        b = c // 4
        sh = c % 4
        x0T = np.ascontiguousarray(x0[b].T)  # [768, 1024]
        in_maps.append(
            {
                "x0T": x0T,
                "lnfp": lnfp,
                "qkb": qkb, "vbr": vbr, "pbr": pbr, "fcb": fcb, "fpbr": fpbr,
                "aw": aw, "pw": pw, "fw": fw, "fpw": fpw,
                "wteT": np.ascontiguousarray(wteT[:, sh * VS : (sh + 1) * VS]),
                "msk": msk,
            }
        )
    return in_maps


def _run(inputs, L, VS, trace=False):
    in_maps = _prepare(inputs, L, VS)
    m0 = in_maps[0]
    no_bias = bool(
        not np.any(m0["vbr"]) and not np.any(m0["pbr"]) and not np.any(m0["fpbr"])
    )
    nc = _get_nc(L, VS, no_bias=no_bias)
    res = run_bass_kernel_spmd(
        nc, in_maps, core_ids=list(range(NCORE)), trace=trace
    )
    # core c holds batch c//4, vocab shard c%4: concat shards per batch
    outs = [res.results[c]["out"] for c in range(NCORE)]
    logits = np.stack(
        [np.concatenate(outs[4 * b : 4 * b + 4], axis=1)[:, :V] for b in range(B)]
    )
    return np.ascontiguousarray(logits), res


def kernel(**inputs) -> np.ndarray:
    trace = bool(os.environ.get("_KERNEL_TRACE"))
    logits, _ = _run(inputs, L_FULL, VSH, trace=trace)
    return logits


if __name__ == "__main__":
    pass
